# revision 83
# baseline (speedup 1.0000x reference)
"""GAT (2-layer, 8-head) Trainium2 Bass kernel, 8-core row-parallel SPMD.

Sharding: nodes (rows of x / adj) split across 8 cores, 512 rows each. Each
core computes its rows' attention against all 4096 nodes; small weights
replicated. Attention math runs transposed (source j on partitions, my row i
on the free dim) so aggregation maps onto the PE without transposing the big
attention matrix.

Key algebra (per head): unnormalized weight
  p[j,i] = adj * exp(leaky_relu(s_i + d_j)) = adj * max(e^{0.2 d} * e^{-0.8 s},
           e^{d}) * e^{0.8 s_i}
The per-row factor e^{0.8 s_i} cancels in the softmax and |s|,|d| < 6 for this
data so no max-subtraction stabilization is needed. Per (j-chunk, i) two
elementwise ops remain, assigned per-chunk to one of three engine patterns
(DVE-only / ACT+DVE+extra-PE / DVE+GPSIMD) chosen by an engine-balance LP:
  m08 = (embs * e02d_j) max e1d_j        then    pt = m08 * adjT
All hot elementwise ops are bf16 (DVE 4x mode for tensor_scalar, 2x for
tensor_tensor); matmuls are bf16 or f32r (1 cycle/row).

Hot-loop epilogue: denominator broadcast via GPSIMD partition_broadcast,
ELU's min via two ACT relu/exp ops, and the layer-2 Wh2 + dst2 + src2 rows
come from a single f32r matmul with a [W_out | v2d | 0.. | v2s] packed lhsT
(f32r matmul outputs must start at PSUM partition 0; row 96 keeps the src2
read 32-aligned). Head 0's Wh-precompute (whca/whce) is emitted inside its
own chunk loop so phase A pipelines with the first head.

Cross-core communication: the layer-2 Wh2 gather is pipelined in 2 row-block
slices (transpose -> DMA -> AllGather -> return DMA -> per-slice exp prep),
with L2 attention chunks processed in slice-arrival order; the gathered f32r
payload is used directly as matmul lhsT (no post-gather conversion). BatchNorm
ships one AllReduce of [sum, sum_sq] with the stats fused into the
normalization ops via stt accum_out; a dummy sqrt preloads the ACT sqrt table
during the collective round trip.
"""

import numpy as np
from contextlib import ExitStack

import concourse.bass as bass
import concourse.bacc as bacc
import concourse.tile as tile
from concourse import mybir
from concourse.bass_utils import run_bass_kernel_spmd

F32 = mybir.dt.float32
F32R = mybir.dt.float32r
BF16 = mybir.dt.bfloat16
I32 = mybir.dt.int32
AF = mybir.ActivationFunctionType
ALU = mybir.AluOpType

N_CORES = 8
N = 4096
NIN = 128
NHID = 64
NOUT = 64
H = 8
MY = N // N_CORES          # 512 rows per core
NJC = N // 128             # 32 j-chunks
NIB = MY // 128            # 4 row blocks per core
ALPHA = 0.2
EPS = 1e-5

# L1 per-chunk engine patterns (from LP over measured per-op costs):
#  'a' = DVE ts (.19) + DVE tt (.33)
#  'm' = ACT relu(embs*e02d - e1d) (.61) + DVE tt (.33) + extra PE matmul of
#        the e1d branch against raw adjT with e1d-prescaled weights (exact:
#        max(A,B) = B + relu(A-B))
#  'q' = DVE ts (.19) + Pool tt (1.11)
N_M, N_Q = 17, 9

# L2 (f32r) per-chunk patterns: 'x' = ACT mul + DVE stt; 's' = Pool ts +
# Pool tt (no DVE/ACT).  s-rows within each arrival slice:
L2_S_ROWS = (2, 5)


def _make_pattern():
    pat = ['a'] * NJC
    ms = [int(round(k * NJC / N_M)) for k in range(N_M)]
    for i in ms:
        pat[i] = 'm'
    free = [i for i in range(NJC) if pat[i] == 'a']
    qs = [free[int(round(k * len(free) / N_Q))] for k in range(N_Q)]
    for i in qs:
        pat[i] = 'q'
    return pat


PATTERN = _make_pattern()
M_JCS = [i for i in range(NJC) if PATTERN[i] == 'm']
M_IDX = {jc: k for k, jc in enumerate(M_JCS)}
# head 0 runs while phase A keeps ACT busy producing Wh: use fewer
# ACT-residual chunks there (m-subset keeps whce indexing valid)
PATTERN0 = list(PATTERN)
for _k, _jc in enumerate(M_JCS):
    if _k % 2 == 1:
        PATTERN0[_jc] = 'a'

_CACHED = {}


def build_program(sim=False):
    nd = 1 if sim else N_CORES
    nc = bacc.Bacc("TRN2", target_bir_lowering=False, debug=False,
                   num_devices=nd)

    d = {}
    d["adjt"] = nc.dram_tensor("adjt", [128, NJC * MY], BF16,
                               kind="ExternalInput")
    d["xt"] = nc.dram_tensor("xt", [NIN, N], BF16, kind="ExternalInput")
    d["xmt"] = nc.dram_tensor("xmt", [NIN, MY], BF16, kind="ExternalInput")
    d["blobf"] = nc.dram_tensor("blobf", [128, 624], F32,
                                kind="ExternalInput")
    d["blobb"] = nc.dram_tensor("blobb", [128, 528], BF16,
                                kind="ExternalInput")
    d["y"] = nc.dram_tensor("y", [NOUT, MY], F32, kind="ExternalOutput")

    with tile.TileContext(nc) as tc:
        with ExitStack() as ctx:
            _build_body(nc, tc, ctx, d, sim=sim)
    nc.compile()
    return nc


FAKE_COLLECTIVES = False


def _build_body(nc, tc, ctx, d, sim):
    def collective(kind, op, in_ap, out_ap):
        if sim or FAKE_COLLECTIVES:
            nc.sync.dma_start(out=out_ap[0] if kind == "AllGather"
                              else out_ap.opt(), in_=in_ap.opt())
        else:
            nc.gpsimd.collective_compute(
                kind, op, replica_groups=[list(range(N_CORES))],
                ins=[in_ap.opt()], outs=[out_ap.opt()])

    consts = ctx.enter_context(tc.tile_pool(name="consts", bufs=1))
    persist = ctx.enter_context(tc.tile_pool(name="persist", bufs=1))
    dram = ctx.enter_context(tc.tile_pool(name="dram", bufs=1, space="DRAM"))

    ones_row = consts.tile([1, 128], F32)
    nc.gpsimd.memset(ones_row, 1.0)
    onesb = consts.tile([1, NHID], BF16)
    nc.gpsimd.memset(onesb, 1.0)
    ones512b = consts.tile([1, MY], F32)
    nc.gpsimd.memset(ones512b, 1.0)

    # ---------------- persistent intermediates ----------------
    adjT = persist.tile([128, NJC, MY], BF16)   # adj[i, 128*jc+jp], bf16
    e02dc = persist.tile([128, NJC, H], F32)    # exp(0.2*dst)
    e1dc = persist.tile([128, NJC, H], F32)     # exp(dst)
    ne1dc = persist.tile([128, NJC, H], F32)    # -exp(dst)
    embsb = persist.tile([128, H, MY], BF16)    # exp(-0.8*src) bcast
    hcatT = persist.tile([128, 4, MY], F32R)    # layer-1 output (transposed)
    whca = persist.tile([128, NJC, H, NHID + 1], BF16)  # Wh lhsT + ones col
    whce = persist.tile([128, len(M_JCS), H, NHID + 1], BF16)  # Wh*e1d lhsT

    hot = ctx.enter_context(tc.tile_pool(name="hot", bufs=10))
    hotf = ctx.enter_context(tc.tile_pool(name="hotf", bufs=4))
    agg = ctx.enter_context(tc.tile_pool(name="agg", bufs=3, space="PSUM"))
    post = ctx.enter_context(tc.tile_pool(name="post", bufs=2))

    # ============ PHASE A: x-side precompute =============================
    paT = ctx.enter_context(tc.tile_pool(name="paT", bufs=1))
    whp = ctx.enter_context(tc.tile_pool(name="whp", bufs=2, space="PSUM"))
    stg = ctx.enter_context(tc.tile_pool(name="stg", bufs=2))
    srcT = persist.tile([H, MY], F32)
    srcE = persist.tile([1, H * MY], BF16)
    st01 = persist.tile([1, 2 * MY], F32)
    with ExitStack() as actx:
        pa = actx.enter_context(tc.tile_pool(name="pa", bufs=4))
        whpb = actx.enter_context(tc.tile_pool(name="whpb", bufs=1,
                                               space="PSUM"))

        xT = paT.tile([128, N], BF16)
        nc.sync.dma_start(out=xT, in_=d["xt"].ap())
        blobb = consts.tile([128, 528], BF16)
        nc.sync.dma_start(out=blobb, in_=d["blobb"].ap())
        xmT = paT.tile([128, MY], BF16)
        nc.sync.dma_start(out=xmT, in_=d["xmt"].ap())
        blobf = consts.tile([128, 624], F32)
        nc.sync.dma_start(out=blobf, in_=d["blobf"].ap())
        ident = blobf[:, 0:128]
        gb = blobf[0:NOUT, 128:130]
        c2rep = blobf[:, 130:133]   # [.2*c2d, c2d, -.8*c2s] replicated
        # wv2[p, t, 0:64] = W_out chunk t; col 64 = v2d, cols 65:96 zero,
        # col 96 = v2s (row 96 is 32-aligned for the src2 PSUM read)
        wv2 = blobf[:, 133:521].rearrange("p (t d) -> p t d", t=4)
        wcsn97 = blobf[0:1, 521:618]   # [-colsum(W_out) | zeros]
        wallb = blobb[:, 0:512]
        vallb = blobb[:, 512:528]

       
        # dst[j,h] = x[j,:] @ (W_h @ a_dst_h); e02d = e^{0.2 dst}, e1d = e^dst
        dstps = whpb.tile([128, NJC, H], F32, tag="dstps")
        for jc in range(NJC):
            nc.tensor.matmul(dstps[:, jc, :], xT[:, 128 * jc:128 * (jc + 1)],
                             vallb[:, 0:H], start=True, stop=True)
        nc.scalar.activation(e02dc, dstps, AF.Exp, scale=0.2)
        nc.scalar.activation(e1dc, dstps, AF.Exp)
        nc.vector.tensor_scalar_mul(ne1dc, e1dc, -1.0)

        # f32r copy of the packed layer-2 weights (engine-rounded lhsT)
        wv2r = persist.tile([128, 4, 97], F32R)
        nc.vector.tensor_copy(wv2r, wv2)

        # src[i,h] for my rows -> srcT [8, MY] (per-head exp + broadcast
        # happen inside the hot loop)
        for t in range(NIB):
            ps = whp.tile([128, 512], F32, tag="ps")
            nc.tensor.matmul(ps[:, 0:H], xmT[:, 128 * t:128 * (t + 1)],
                             vallb[:, H:2 * H], start=True, stop=True)
            srcblk = pa.tile([128, H], F32, tag="srcblk")
            nc.scalar.copy(srcblk, ps[:, 0:H])
            ps2 = whp.tile([128, 512], F32, tag="ps")
            nc.tensor.transpose(ps2[0:H, 0:128], srcblk,
                                ident[0:128, 0:128])
            nc.scalar.copy(srcT[:, 128 * t:128 * (t + 1)], ps2[0:H, 0:128])
        # adjT load in 8 slices so head 0's first chunks unblock early;
        # the head-0/1 src staging DMAs slot in after slice 1 so embsb is
        # ready ~2us sooner than behind the whole adjT stream
        for q in range(8):
            nc.sync.dma_start(
                out=adjT[:, 4 * q:4 * (q + 1), :],
                in_=d["adjt"].ap().rearrange("p (jc i) -> p jc i", jc=NJC)
                [:, 4 * q:4 * (q + 1), :])
            if q == 2:
                for h01 in range(2):
                    nc.sync.dma_start(
                        out=st01[0:1, h01 * MY:(h01 + 1) * MY],
                        in_=srcT[h01:h01 + 1, :])
        nc.gpsimd.memset(whca[:, :, :, NHID:NHID + 1], 1.0)

    def head_prefix(h):
        # embsb[h] = exp(-0.8*src_h) broadcast across partitions (row h is
        # staged to partition 0 by DMA: engine reads must be 32-aligned).
        # Heads 0-1 use the pre-staged st01 rows whose DMAs were slotted
        # into the adjT stream so the ramp isn't queue-blocked.
        if h < 2:
            stage = st01[0:1, h * MY:(h + 1) * MY]
        else:
            stage = stg.tile([1, MY], F32, tag="stage")
            nc.sync.dma_start(out=stage, in_=srcT[h:h + 1, :])
        nc.scalar.activation(srcE[0:1, h * MY:(h + 1) * MY], stage,
                             AF.Exp, scale=-0.8)
        nc.gpsimd.partition_broadcast(embsb[:, h, :],
                                      srcE[0:1, h * MY:(h + 1) * MY])

    def whca_chunk(jc):
        # Wh for all heads -> whca[:, jc, h, 0:64] (bf16); e1d-scaled copy
        # for 'm' chunks (emitted inside head 0's loop for pipelining)
        ps = whp.tile([128, 512], F32, tag="ps")
        nc.tensor.matmul(ps, xT[:, 128 * jc:128 * (jc + 1)], wallb,
                         start=True, stop=True)
        dst_ap = whca[:, jc, :, 0:NHID]
        src_ap = ps.rearrange("p (h d) -> p h d", h=H)
        if jc % 3 == 2:
            nc.vector.tensor_copy(dst_ap, src_ap)
        else:
            nc.scalar.copy(dst_ap, src_ap)
        if jc in M_IDX:
            mi = M_IDX[jc]
            for h in range(H):
                nc.vector.tensor_scalar(whce[:, mi, h, :],
                                        whca[:, jc, h, :],
                                        e1dc[:, jc, h:h + 1], None,
                                        op0=ALU.mult)

    # ============ attention hot loop (layer 1) ===========================
    def attention(embs_ap, e02col_of, e1col_of, ne1col_of, lhsT_of,
                  lhsTe_of, interleave=None, at_jc=31, pre_chunk=None,
                  pattern=PATTERN):
        aggps = agg.tile([NHID + 1, MY], F32, tag="aggps")
        first = [True]

        def mm(lhsT, rhs, stop=False):
            nc.tensor.matmul(aggps, lhsT, rhs, start=first[0], stop=stop)
            first[0] = False

        for k in range(NJC):
            if k == at_jc and interleave is not None:
                interleave()
            jc = k
            if pre_chunk is not None:
                pre_chunk(jc)
            pat = pattern[jc]
            last = k == NJC - 1
            pt = hot.tile([128, MY], BF16, tag="pt")
            if pat == 'm':
                # e1d branch via PE on raw adjT; relu residual elementwise
                mm(lhsTe_of(jc), adjT[:, jc, :])
                rr = hot.tile([128, MY], BF16, tag="m08")
                nc.scalar.activation(rr, embs_ap, AF.Relu,
                                     scale=e02col_of(jc),
                                     bias=ne1col_of(jc))
                nc.vector.tensor_mul(pt, rr, adjT[:, jc, :])
            else:
                m08 = hot.tile([128, MY], BF16, tag="m08")
                nc.vector.tensor_scalar(m08, embs_ap, e02col_of(jc),
                                        e1col_of(jc),
                                        op0=ALU.mult, op1=ALU.max)
                if pat == 'a':
                    nc.vector.tensor_mul(pt, m08, adjT[:, jc, :])
                else:
                    nc.gpsimd.tensor_mul(pt, m08, adjT[:, jc, :])
            mm(lhsT_of(jc), pt[:], stop=last)
        return aggps

    # ============ PHASE C: layer-1 heads =================================
    # hcatT holds ELU(out1)+1; the -1 is folded into the Wh2 bias matmul
    # (wcsn = -colsum(W_out)).
    p2 = ctx.enter_context(tc.tile_pool(name="p2", bufs=1))
    wh2p = ctx.enter_context(tc.tile_pool(name="wh2p", bufs=1, space="PSUM"))
    # one PSUM bank: rows 0:64 Wh2, row 64 dst2, row 65 src2 (f32r matmul
    # outputs must start at partition 0)
    wh2ps = wh2p.tile([97, MY], F32, tag="wh2")
    nc.tensor.matmul(wh2ps, wcsn97, ones512b, start=True, stop=False)

    def l1_epilogue(h, aggps, c0=0, c1=MY):
        w = c1 - c0
        linv = post.tile([1, MY], F32, tag="linv")
        nc.vector.reciprocal(linv[:, 0:w], aggps[NHID:NHID + 1, c0:c1])
        lbs = post.tile([NHID, MY], F32, tag="lbs")
        nc.gpsimd.partition_broadcast(lbs[:, 0:w], linv[0:1, 0:w])
        scaled = post.tile([NHID, MY], F32, tag="scaled")
        nc.vector.tensor_mul(scaled[:, 0:w], aggps[0:NHID, c0:c1],
                             lbs[:, 0:w])
        # ELU(x)+1 = max(x,0) + exp(min(x,0)); min via relu(-x) on ACT
        nr = post.tile([NHID, MY], F32, tag="tmp")
        nc.scalar.activation(nr[:, 0:w], scaled[:, 0:w], AF.Relu, scale=-1.0)
        em = post.tile([NHID, MY], F32, tag="tmp")
        nc.scalar.activation(em[:, 0:w], nr[:, 0:w], AF.Exp, scale=-1.0)
        dst_rows = hcatT[64 * (h % 2):64 * (h % 2) + NHID, h // 2, c0:c1]
        nc.vector.scalar_tensor_tensor(dst_rows, scaled[:, 0:w], 0.0,
                                       em[:, 0:w], op0=ALU.max, op1=ALU.add)
        if h % 2 == 1:
            t = h // 2
            nc.tensor.matmul(wh2ps[:, c0:c1], wv2r[:, t, :],
                             hcatT[:, t, c0:c1], start=False, stop=(t == 3))

    pending = None
    head_prefix(0)
    head_prefix(1)
    for h in range(H):
        def cb(p=pending, h=h):
            if p is not None:
                l1_epilogue(*p)
            if h + 2 < H:
                head_prefix(h + 2)
        aggps = attention(
            embsb[:, h, :],
            lambda jc, h=h: e02dc[:, jc, h:h + 1],
            lambda jc, h=h: e1dc[:, jc, h:h + 1],
            lambda jc, h=h: ne1dc[:, jc, h:h + 1],
            lambda jc, h=h: whca[:, jc, h, :],
            lambda jc, h=h: whce[:, M_IDX[jc], h, :], interleave=cb,
            pre_chunk=(whca_chunk if h == 0 else None),
            pattern=(PATTERN0 if h == 0 else PATTERN))
        pending = (h, aggps)
    # ============ PHASE D: output attention layer ========================
    # The final head's epilogue, Wh2 matmul, payload transpose and gather
    # run per column half so slice 0's AllGather flies while half B still
    # computes.  wh2Tme rows 0:64 = Wh2 (f32r); row 64 = raw dst2 (c2 bias
    # folded into the post-gather exps).  The L2 attention runs in f32r:
    # batchnorm divides by a tiny cross-node std, amplifying L2-side noise,
    # so bf16 is not enough here.
    p2p = ctx.enter_context(tc.tile_pool(name="p2p", bufs=1, space="PSUM"))
    wh2Tme = p2.tile([NOUT + 1, MY], F32)
    payload = p2.tile([128, NIB, NOUT + 1], F32R)
    cc_in = dram.tile([2, 256, NOUT + 1], F32R)
    cc_out = dram.tile([2, N_CORES, 256, NOUT + 1], F32R)
    h7, agg7 = pending
    for s in range(2):
        c0, c1 = 256 * s, 256 * s + 256
        l1_epilogue(h7, agg7, c0, c1)
        nc.vector.tensor_copy(wh2Tme[:, c0:c1], wh2ps[0:NOUT + 1, c0:c1])
        for tl in range(2):
            t = 2 * s + tl
            pps = p2p.tile([128, 512], F32, tag="pp2")
            nc.tensor.transpose(pps[:, 0:NOUT + 1],
                                wh2Tme[:, 128 * t:128 * (t + 1)],
                                ident[0:NOUT + 1, 0:NOUT + 1])
            if tl:
                nc.scalar.copy(payload[:, t, :], pps[:, 0:NOUT + 1])
            else:
                nc.vector.tensor_copy(payload[:, t, :], pps[:, 0:NOUT + 1])
        nc.sync.dma_start(
            out=cc_in[s].rearrange("(t p) d -> p t d", t=2),
            in_=payload[:, 2 * s:2 * s + 2, :])
        collective("AllGather", ALU.bypass, cc_in[s], cc_out[s])

    # local-only src2 path (off the gather critical path)
    s2r = p2.tile([1, MY], F32R)
    nc.scalar.activation(s2r, wh2ps[96:97, :], AF.Exp,
                         scale=-0.8, bias=c2rep[0:1, 2:3])
    embsb2 = p2.tile([128, MY], F32R)
    nc.gpsimd.partition_broadcast(embsb2, s2r[0:1, :])

    # wh2aug[jp, jc=4r+t, 0:64] = Wh2 lhsT, col 64 = dst2 raw; f32r straight
    # from DMA (PE rounds on read; no conversion copies needed)
    wh2aug = p2.tile([128, NJC, NOUT + 1], F32R)
    wh2aug_r = wh2aug.rearrange("p (r t) d -> p r t d", t=NIB)
    e02d2 = p2.tile([128, NIB, 8], F32)
    e1d2 = p2.tile([128, NIB, 8], F32)
    for s in range(2):
        for tl in range(2):
            t = 2 * s + tl
            nc.scalar.dma_start(
                out=wh2aug_r[:, :, t, :],
                in_=cc_out[s][:, 128 * tl:128 * (tl + 1), :]
                .rearrange("r p d -> p r d"))
        dcol = wh2aug_r[:, :, 2 * s:2 * s + 2, NOUT:NOUT + 1]
        nc.scalar.activation(
            e02d2[:, 2 * s:2 * s + 2, :],
            dcol.rearrange("p r t one -> p t (r one)"),
            AF.Exp, scale=0.2, bias=c2rep[:, 0:1])
        nc.scalar.activation(
            e1d2[:, 2 * s:2 * s + 2, :],
            dcol.rearrange("p r t one -> p t (r one)"),
            AF.Exp, bias=c2rep[:, 1:2])
        # ones column for the denominator row of every chunk of this slice
        nc.vector.tensor_scalar(dcol, dcol, 0.0, 1.0,
                                op0=ALU.mult, op1=ALU.add)

    # L2 attention: chunks in slice-arrival order; pattern 'x' (ACT mul +
    # DVE stt) or 's' (Pool ts + Pool tt, no DVE/ACT)
    aggps2 = agg.tile([NHID + 1, MY], F32, tag="aggps")
    n_emitted = [0]
    for s in range(2):
        srows = L2_S_ROWS + ((7,) if s == 0 else ())
        xs = [(r, t) for t in (2 * s, 2 * s + 1)
              for r in range(N_CORES) if r not in srows]
        ss = [(r, t) for t in (2 * s, 2 * s + 1)
              for r in range(N_CORES) if r in srows]
        pairs = []
        for i in range(len(xs)):
            if i % 3 == 0 and ss:
                pairs.append(ss.pop(0))
            pairs.append(xs[i])
        pairs.extend(ss)
        for r, t in pairs:
            jc = 4 * r + t
            k = n_emitted[0]
            n_emitted[0] += 1
            last = k == NJC - 1
            pt2 = hotf.tile([128, MY], F32R, tag="pt2")
            if r in L2_S_ROWS:
                m08f = hotf.tile([128, MY], F32, tag="rr")
                nc.gpsimd.tensor_scalar(m08f, embsb2, e02d2[:, t, r:r + 1],
                                        e1d2[:, t, r:r + 1],
                                        op0=ALU.mult, op1=ALU.max)
                nc.gpsimd.tensor_mul(pt2, m08f, adjT[:, jc, :])
            else:
                a2t = hotf.tile([128, MY], F32R, tag="rr")
                nc.scalar.mul(a2t, embsb2, e02d2[:, t, r:r + 1])
                nc.vector.scalar_tensor_tensor(pt2, a2t, e1d2[:, t, r:r + 1],
                                               adjT[:, jc, :],
                                               op0=ALU.max, op1=ALU.mult)
            nc.tensor.matmul(aggps2, wh2aug[:, jc, :], pt2[:],
                             start=(k == 0), stop=last)
    # sqrt-table preload: epst = sqrt(EPS^2) is a real set-3 ACT op whose
    # input is ready at t=0, pulling the table load off the post-collective
    # critical path
    epssq = p2.tile([NOUT, 1], F32)
    nc.gpsimd.memset(epssq, float(EPS) * float(EPS))
    epst = p2.tile([NOUT, 1], F32)
    with tc.high_priority(offset=70):
        nc.scalar.activation(epst, epssq, AF.Sqrt)

    # ============ PHASE E: batchnorm (single AllReduce of [S, S2]) =======
    linv2 = post.tile([1, MY], F32, tag="linv")
    nc.vector.reciprocal(linv2, aggps2[NHID:NHID + 1, :])
    lbs2 = post.tile([NOUT, MY], F32, tag="lbs")
    nc.gpsimd.partition_broadcast(lbs2, linv2[0:1, :])
    bnS = p2.tile([NOUT, 2], F32)
    out2n = p2.tile([NOUT, MY], F32)
    nc.vector.scalar_tensor_tensor(out2n, aggps2[0:NOUT, :], 0.0, lbs2,
                                   op0=ALU.add, op1=ALU.mult,
                                   accum_out=bnS[:, 0:1])
    sq = post.tile([NOUT, MY], F32, tag="tmp")
    nc.vector.scalar_tensor_tensor(sq, out2n, 0.0, out2n,
                                   op0=ALU.add, op1=ALU.mult,
                                   accum_out=bnS[:, 1:2])
    bn_in = dram.tile([NOUT, 2], F32)
    bn_out = dram.tile([NOUT, 2], F32)
    nc.scalar.dma_start(out=bn_in, in_=bnS)
    collective("AllReduce", ALU.add, bn_in, bn_out)
    bnG = p2.tile([NOUT, 2], F32)
    nc.scalar.dma_start(out=bnG, in_=bn_out)
    negmu = p2.tile([NOUT, 1], F32)
    nc.vector.tensor_scalar_mul(negmu, bnG[:, 0:1], -1.0 / N)
    mu2 = p2.tile([NOUT, 1], F32)
    nc.vector.tensor_mul(mu2, negmu, negmu)
    var = p2.tile([NOUT, 1], F32)
    nc.vector.scalar_tensor_tensor(var, bnG[:, 1:2], 1.0 / N, mu2,
                                   op0=ALU.mult, op1=ALU.subtract)

    sd = p2.tile([NOUT, 1], F32)
    nc.scalar.activation(sd, var, AF.Sqrt, bias=epst[:, 0:1])
    rstd = p2.tile([NOUT, 1], F32)
    nc.vector.reciprocal(rstd, sd)
    scale = p2.tile([NOUT, 1], F32)
    nc.vector.tensor_mul(scale, rstd, gb[:, 0:1])
    shift = p2.tile([NOUT, 1], F32)
    nc.vector.tensor_mul(shift, negmu, scale)
    nc.vector.tensor_add(shift, shift, gb[:, 1:2])

    finT = p2.tile([NOUT, MY], F32)
    nc.scalar.activation(finT, out2n, AF.Tanh, bias=shift[:, 0:1],
                         scale=scale[:, 0:1])
    nc.sync.dma_start(out=d["y"].ap(), in_=finT)


def _prep_inputs(x, adj, W_heads, a_heads, W_out, a_out, gamma, beta):
    """Host-side packing of the small weights + per-core sharding."""
    import ml_dtypes
    BF = ml_dtypes.bfloat16
    x = np.ascontiguousarray(np.asarray(x, dtype=np.float32))
    adj = np.asarray(adj, dtype=np.int32)
    W_heads = np.asarray(W_heads, dtype=np.float32)
    a_heads = np.asarray(a_heads, dtype=np.float32)
    W_out = np.asarray(W_out, dtype=np.float32)
    a_out = np.asarray(a_out, dtype=np.float32)

    wall = np.ascontiguousarray(
        W_heads.transpose(1, 0, 2).reshape(NIN, H * NHID))
    vall = np.zeros((NIN, 2 * H), np.float32)
    for h in range(H):
        vall[:, h] = W_heads[h] @ a_heads[h, NHID:]        # dst direction
        vall[:, H + h] = W_heads[h] @ a_heads[h, :NHID]    # src direction
    # woutt[p, t*64+d] = W_out[t*128+p, d]  (lhsT chunks)
    woutt = W_out.reshape(4, 128, NOUT).transpose(1, 0, 2).reshape(128, -1)
    wcs = W_out.sum(axis=0)
    v2d = W_out @ a_out[NOUT:]          # dst direction, [512]
    v2s = W_out @ a_out[:NOUT]          # src direction
    v2t = (np.stack([v2d, v2s], axis=1).reshape(4, 128, 2)
           .transpose(1, 0, 2).reshape(128, 8))
    eye = np.eye(128, dtype=np.float32)
    c2d = -(wcs @ a_out[NOUT:])
    c2s = -(wcs @ a_out[:NOUT])

    blobf = np.zeros((128, 624), np.float32)
    blobf[:, 0:128] = eye
    blobf[0:NOUT, 128] = np.asarray(gamma, np.float32)
    blobf[0:NOUT, 129] = np.asarray(beta, np.float32)
    blobf[:, 130] = 0.2 * c2d
    blobf[:, 131] = c2d
    blobf[:, 132] = -0.8 * c2s
    # packed [W_out | v2d | 0.. | v2s] per 128-row chunk t
    wv2 = np.zeros((4, 128, 97), np.float32)
    wv2[:, :, 0:NOUT] = W_out.reshape(4, 128, NOUT)
    wv2[:, :, NOUT] = v2d.reshape(4, 128)
    wv2[:, :, 96] = v2s.reshape(4, 128)
    blobf[:, 133:521] = wv2.transpose(1, 0, 2).reshape(128, -1)
    blobf[0, 521:585] = -wcs
    blobb = np.zeros((128, 528), np.float32)
    blobb[:, 0:512] = wall
    blobb[:, 512:528] = vall
    blobb = blobb.astype(BF)

    xt = np.ascontiguousarray(x.T).astype(BF)
    adjbf = adj.astype(BF)
    shared = {"xt": xt, "blobf": blobf, "blobb": blobb}
    in_maps = []
    for c in range(N_CORES):
        m = dict(shared)
        m["xmt"] = np.ascontiguousarray(x[MY * c:MY * (c + 1)].T).astype(BF)
        # adjt[p, jc*MY + i] = adj[MY*c + i, 128*jc + p]
        m["adjt"] = np.ascontiguousarray(
            adjbf[MY * c:MY * (c + 1)].T.reshape(NJC, 128, MY)
            .transpose(1, 0, 2).reshape(128, NJC * MY))
        in_maps.append(m)
    return in_maps


def kernel(x, adj, W_heads, a_heads, W_out, a_out, gamma, beta, **kw):
    if "nc" not in _CACHED:
        _CACHED["nc"] = build_program()
    nc = _CACHED["nc"]
    in_maps = _prep_inputs(x, adj, W_heads, a_heads, W_out, a_out, gamma, beta)
    res = run_bass_kernel_spmd(nc, in_maps, core_ids=list(range(N_CORES)),
                               **kw)
    _CACHED["last_res"] = res
    out = np.concatenate([res.results[c]["y"].T for c in range(N_CORES)],
                         axis=0)
    return np.ascontiguousarray(out)


# revision 84
# speedup vs baseline: 1.0118x; 1.0118x over previous
"""GAT (2-layer, 8-head) Trainium2 Bass kernel, 8-core row-parallel SPMD.

Sharding: nodes (rows of x / adj) split across 8 cores, 512 rows each. Each
core computes its rows' attention against all 4096 nodes; small weights
replicated. Attention math runs transposed (source j on partitions, my row i
on the free dim) so aggregation maps onto the PE without transposing the big
attention matrix.

Key algebra (per head): unnormalized weight
  p[j,i] = adj * exp(leaky_relu(s_i + d_j)) = adj * max(e^{0.2 d} * e^{-0.8 s},
           e^{d}) * e^{0.8 s_i}
The per-row factor e^{0.8 s_i} cancels in the softmax and |s|,|d| < 6 for this
data so no max-subtraction stabilization is needed. Per (j-chunk, i) two
elementwise ops remain, assigned per-chunk to one of three engine patterns
(DVE-only / ACT+DVE+extra-PE / DVE+GPSIMD) chosen by an engine-balance LP:
  m08 = (embs * e02d_j) max e1d_j        then    pt = m08 * adjT
All hot elementwise ops are bf16 (DVE 4x mode for tensor_scalar, 2x for
tensor_tensor); matmuls are bf16 or f32r (1 cycle/row).

Hot-loop epilogue: denominator broadcast via GPSIMD partition_broadcast,
ELU's min via two ACT relu/exp ops, and the layer-2 Wh2 + dst2 + src2 rows
come from a single f32r matmul with a [W_out | v2d | 0.. | v2s] packed lhsT
(f32r matmul outputs must start at PSUM partition 0; row 96 keeps the src2
read 32-aligned). Head 0's Wh-precompute (whca/whce) is emitted inside its
own chunk loop so phase A pipelines with the first head.

Cross-core communication: the layer-2 Wh2 gather is pipelined in 2 row-block
slices (transpose -> DMA -> AllGather -> return DMA -> per-slice exp prep),
with L2 attention chunks processed in slice-arrival order; the gathered f32r
payload is used directly as matmul lhsT (no post-gather conversion). BatchNorm
ships one AllReduce of [sum, sum_sq] with the stats fused into the
normalization ops via stt accum_out; a dummy sqrt preloads the ACT sqrt table
during the collective round trip.
"""

import numpy as np
from contextlib import ExitStack

import concourse.bass as bass
import concourse.bacc as bacc
import concourse.tile as tile
from concourse import mybir
from concourse.bass_utils import run_bass_kernel_spmd

F32 = mybir.dt.float32
F32R = mybir.dt.float32r
BF16 = mybir.dt.bfloat16
I32 = mybir.dt.int32
AF = mybir.ActivationFunctionType
ALU = mybir.AluOpType

N_CORES = 8
N = 4096
NIN = 128
NHID = 64
NOUT = 64
H = 8
MY = N // N_CORES          # 512 rows per core
NJC = N // 128             # 32 j-chunks
NIB = MY // 128            # 4 row blocks per core
ALPHA = 0.2
EPS = 1e-5

# L1 per-chunk engine patterns (from LP over measured per-op costs):
#  'a' = DVE ts (.19) + DVE tt (.33)
#  'm' = ACT relu(embs*e02d - e1d) (.61) + DVE tt (.33) + extra PE matmul of
#        the e1d branch against raw adjT with e1d-prescaled weights (exact:
#        max(A,B) = B + relu(A-B))
#  'q' = DVE ts (.19) + Pool tt (1.11)
N_M, N_Q = 17, 9

# L2 (f32r) per-chunk patterns: 'x' = ACT mul + DVE stt; 's' = Pool ts +
# Pool tt (no DVE/ACT).  s-rows within each arrival slice:
L2_S_ROWS = (2, 5)


def _make_pattern():
    pat = ['a'] * NJC
    ms = [int(round(k * NJC / N_M)) for k in range(N_M)]
    for i in ms:
        pat[i] = 'm'
    free = [i for i in range(NJC) if pat[i] == 'a']
    qs = [free[int(round(k * len(free) / N_Q))] for k in range(N_Q)]
    for i in qs:
        pat[i] = 'q'
    return pat


PATTERN = _make_pattern()
M_JCS = [i for i in range(NJC) if PATTERN[i] == 'm']
M_IDX = {jc: k for k, jc in enumerate(M_JCS)}
# head 0 runs while phase A keeps ACT busy producing Wh: use fewer
# ACT-residual chunks there (m-subset keeps whce indexing valid)
PATTERN0 = list(PATTERN)
for _k, _jc in enumerate(M_JCS):
    if _k % 2 == 1:
        PATTERN0[_jc] = 'a'

_CACHED = {}


def build_program(sim=False):
    nd = 1 if sim else N_CORES
    nc = bacc.Bacc("TRN2", target_bir_lowering=False, debug=False,
                   num_devices=nd)

    d = {}
    d["adjt"] = nc.dram_tensor("adjt", [128, NJC * MY], BF16,
                               kind="ExternalInput")
    d["xt"] = nc.dram_tensor("xt", [NIN, N], BF16, kind="ExternalInput")
    d["xmt"] = nc.dram_tensor("xmt", [NIN, MY], BF16, kind="ExternalInput")
    d["blobf"] = nc.dram_tensor("blobf", [128, 624], F32,
                                kind="ExternalInput")
    d["blobb"] = nc.dram_tensor("blobb", [128, 528], BF16,
                                kind="ExternalInput")
    d["y"] = nc.dram_tensor("y", [NOUT, MY], F32, kind="ExternalOutput")

    with tile.TileContext(nc) as tc:
        with ExitStack() as ctx:
            _build_body(nc, tc, ctx, d, sim=sim)
    nc.compile()
    return nc


FAKE_COLLECTIVES = False


def _build_body(nc, tc, ctx, d, sim):
    def collective(kind, op, in_ap, out_ap):
        if sim or FAKE_COLLECTIVES:
            nc.sync.dma_start(out=out_ap[0] if kind == "AllGather"
                              else out_ap.opt(), in_=in_ap.opt())
        else:
            nc.gpsimd.collective_compute(
                kind, op, replica_groups=[list(range(N_CORES))],
                ins=[in_ap.opt()], outs=[out_ap.opt()])

    consts = ctx.enter_context(tc.tile_pool(name="consts", bufs=1))
    persist = ctx.enter_context(tc.tile_pool(name="persist", bufs=1))
    dram = ctx.enter_context(tc.tile_pool(name="dram", bufs=1, space="DRAM"))

    ones_row = consts.tile([1, 128], F32)
    nc.gpsimd.memset(ones_row, 1.0)
    onesb = consts.tile([1, NHID], BF16)
    nc.gpsimd.memset(onesb, 1.0)
    ones512b = consts.tile([1, MY], F32)
    nc.gpsimd.memset(ones512b, 1.0)

    # ---------------- persistent intermediates ----------------
    adjT = persist.tile([128, NJC, MY], BF16)   # adj[i, 128*jc+jp], bf16
    e02dc = persist.tile([128, NJC, H], F32)    # exp(0.2*dst)
    e1dc = persist.tile([128, NJC, H], F32)     # exp(dst)
    ne1dc = persist.tile([128, NJC, H], F32)    # -exp(dst)
    embsb = persist.tile([128, H, MY], BF16)    # exp(-0.8*src) bcast
    hcatT = persist.tile([128, 4, MY], F32R)    # layer-1 output (transposed)
    whca = persist.tile([128, NJC, H, NHID + 1], BF16)  # Wh lhsT + ones col
    whce = persist.tile([128, len(M_JCS), H, NHID + 1], BF16)  # Wh*e1d lhsT

    hot = ctx.enter_context(tc.tile_pool(name="hot", bufs=10))
    hotf = ctx.enter_context(tc.tile_pool(name="hotf", bufs=4))
    agg = ctx.enter_context(tc.tile_pool(name="agg", bufs=3, space="PSUM"))
    post = ctx.enter_context(tc.tile_pool(name="post", bufs=2))

    # ============ PHASE A: x-side precompute =============================
    paT = ctx.enter_context(tc.tile_pool(name="paT", bufs=1))
    whp = ctx.enter_context(tc.tile_pool(name="whp", bufs=2, space="PSUM"))
    stg = ctx.enter_context(tc.tile_pool(name="stg", bufs=2))
    srcT = persist.tile([H, MY], F32)
    srcE = persist.tile([1, H * MY], BF16)
    st01 = persist.tile([1, 2 * MY], F32)
    with ExitStack() as actx:
        pa = actx.enter_context(tc.tile_pool(name="pa", bufs=4))
        whpb = actx.enter_context(tc.tile_pool(name="whpb", bufs=1,
                                               space="PSUM"))

        xT = paT.tile([128, N], BF16)
        nc.sync.dma_start(out=xT, in_=d["xt"].ap())
        blobb = consts.tile([128, 528], BF16)
        nc.sync.dma_start(out=blobb, in_=d["blobb"].ap())
        xmT = paT.tile([128, MY], BF16)
        nc.sync.dma_start(out=xmT, in_=d["xmt"].ap())
        blobf = consts.tile([128, 624], F32)
        nc.sync.dma_start(out=blobf, in_=d["blobf"].ap())
        ident = blobf[:, 0:128]
        gb = blobf[0:NOUT, 128:130]
        c2rep = blobf[:, 130:133]   # [.2*c2d, c2d, -.8*c2s] replicated
        # wv2[p, t, 0:64] = W_out chunk t; col 64 = v2d, cols 65:96 zero,
        # col 96 = v2s (row 96 is 32-aligned for the src2 PSUM read)
        wv2 = blobf[:, 133:521].rearrange("p (t d) -> p t d", t=4)
        wcsn97 = blobf[0:1, 521:618]   # [-colsum(W_out) | zeros]
        wallb = blobb[:, 0:512]
        vallb = blobb[:, 512:528]

       
        # dst[j,h] = x[j,:] @ (W_h @ a_dst_h); e02d = e^{0.2 dst}, e1d = e^dst
        dstps = whpb.tile([128, NJC, H], F32, tag="dstps")
        for jc in range(NJC):
            nc.tensor.matmul(dstps[:, jc, :], xT[:, 128 * jc:128 * (jc + 1)],
                             vallb[:, 0:H], start=True, stop=True)
        nc.scalar.activation(e02dc, dstps, AF.Exp, scale=0.2)
        nc.scalar.activation(e1dc, dstps, AF.Exp)
        nc.vector.tensor_scalar_mul(ne1dc, e1dc, -1.0)

        # f32r copy of the packed layer-2 weights (engine-rounded lhsT)
        wv2r = persist.tile([128, 4, 97], F32R)
        nc.vector.tensor_copy(wv2r, wv2)

        # src[i,h] for my rows -> srcT [8, MY] (per-head exp + broadcast
        # happen inside the hot loop)
        for t in range(NIB):
            ps = whp.tile([128, 512], F32, tag="ps")
            nc.tensor.matmul(ps[:, 0:H], xmT[:, 128 * t:128 * (t + 1)],
                             vallb[:, H:2 * H], start=True, stop=True)
            srcblk = pa.tile([128, H], F32, tag="srcblk")
            nc.scalar.copy(srcblk, ps[:, 0:H])
            ps2 = whp.tile([128, 512], F32, tag="ps")
            nc.tensor.transpose(ps2[0:H, 0:128], srcblk,
                                ident[0:128, 0:128])
            nc.scalar.copy(srcT[:, 128 * t:128 * (t + 1)], ps2[0:H, 0:128])
        # adjT load in 8 slices so head 0's first chunks unblock early;
        # the head-0/1 src staging DMAs slot in after slice 1 so embsb is
        # ready ~2us sooner than behind the whole adjT stream
        for q in range(8):
            nc.sync.dma_start(
                out=adjT[:, 4 * q:4 * (q + 1), :],
                in_=d["adjt"].ap().rearrange("p (jc i) -> p jc i", jc=NJC)
                [:, 4 * q:4 * (q + 1), :])
            if q == 2:
                for h01 in range(2):
                    nc.sync.dma_start(
                        out=st01[0:1, h01 * MY:(h01 + 1) * MY],
                        in_=srcT[h01:h01 + 1, :])
        nc.gpsimd.memset(whca[:, :, :, NHID:NHID + 1], 1.0)

    def head_prefix(h):
        # embsb[h] = exp(-0.8*src_h) broadcast across partitions (row h is
        # staged to partition 0 by DMA: engine reads must be 32-aligned).
        # Heads 0-1 use the pre-staged st01 rows whose DMAs were slotted
        # into the adjT stream so the ramp isn't queue-blocked.
        if h < 2:
            stage = st01[0:1, h * MY:(h + 1) * MY]
        else:
            stage = stg.tile([1, MY], F32, tag="stage")
            nc.sync.dma_start(out=stage, in_=srcT[h:h + 1, :])
        nc.scalar.activation(srcE[0:1, h * MY:(h + 1) * MY], stage,
                             AF.Exp, scale=-0.8)
        nc.gpsimd.partition_broadcast(embsb[:, h, :],
                                      srcE[0:1, h * MY:(h + 1) * MY])

    def whca_chunk(jc):
        # Wh for all heads -> whca[:, jc, h, 0:64] (bf16); e1d-scaled copy
        # for 'm' chunks (emitted inside head 0's loop for pipelining)
        ps = whp.tile([128, 512], F32, tag="ps")
        nc.tensor.matmul(ps, xT[:, 128 * jc:128 * (jc + 1)], wallb,
                         start=True, stop=True)
        dst_ap = whca[:, jc, :, 0:NHID]
        src_ap = ps.rearrange("p (h d) -> p h d", h=H)
        if jc % 3 == 2:
            nc.vector.tensor_copy(dst_ap, src_ap)
        else:
            nc.scalar.copy(dst_ap, src_ap)
        if jc in M_IDX:
            mi = M_IDX[jc]
            for h in range(H):
                nc.vector.tensor_scalar(whce[:, mi, h, :],
                                        whca[:, jc, h, :],
                                        e1dc[:, jc, h:h + 1], None,
                                        op0=ALU.mult)

    # ============ attention hot loop (layer 1) ===========================
    def attention(embs_ap, e02col_of, e1col_of, ne1col_of, lhsT_of,
                  lhsTe_of, interleave=None, at_jc=31, pre_chunk=None,
                  pattern=PATTERN):
        aggps = agg.tile([NHID + 1, MY], F32, tag="aggps")
        first = [True]

        def mm(lhsT, rhs, stop=False):
            nc.tensor.matmul(aggps, lhsT, rhs, start=first[0], stop=stop)
            first[0] = False

        for k in range(NJC):
            if k == at_jc and interleave is not None:
                interleave()
            jc = k
            if pre_chunk is not None:
                pre_chunk(jc)
            pat = pattern[jc]
            last = k == NJC - 1
            pt = hot.tile([128, MY], BF16, tag="pt")
            if pat == 'm':
                # e1d branch via PE on raw adjT; relu residual elementwise
                mm(lhsTe_of(jc), adjT[:, jc, :])
                rr = hot.tile([128, MY], BF16, tag="m08")
                nc.scalar.activation(rr, embs_ap, AF.Relu,
                                     scale=e02col_of(jc),
                                     bias=ne1col_of(jc))
                nc.vector.tensor_mul(pt, rr, adjT[:, jc, :])
            else:
                m08 = hot.tile([128, MY], BF16, tag="m08")
                nc.vector.tensor_scalar(m08, embs_ap, e02col_of(jc),
                                        e1col_of(jc),
                                        op0=ALU.mult, op1=ALU.max)
                if pat == 'a':
                    nc.vector.tensor_mul(pt, m08, adjT[:, jc, :])
                else:
                    nc.gpsimd.tensor_mul(pt, m08, adjT[:, jc, :])
            mm(lhsT_of(jc), pt[:], stop=last)
        return aggps

    # ============ PHASE C: layer-1 heads =================================
    # hcatT holds ELU(out1)+1; the -1 is folded into the Wh2 bias matmul
    # (wcsn = -colsum(W_out)).
    p2 = ctx.enter_context(tc.tile_pool(name="p2", bufs=1))
    wh2p = ctx.enter_context(tc.tile_pool(name="wh2p", bufs=1, space="PSUM"))
    # one PSUM bank: rows 0:64 Wh2, row 64 dst2, row 65 src2 (f32r matmul
    # outputs must start at partition 0)
    wh2ps = wh2p.tile([97, MY], F32, tag="wh2")
    nc.tensor.matmul(wh2ps, wcsn97, ones512b, start=True, stop=False)

    def l1_epilogue(h, aggps, c0=0, c1=MY):
        w = c1 - c0
        linv = post.tile([1, MY], F32, tag="linv")
        nc.vector.reciprocal(linv[:, 0:w], aggps[NHID:NHID + 1, c0:c1])
        lbs = post.tile([NHID, MY], F32, tag="lbs")
        nc.gpsimd.partition_broadcast(lbs[:, 0:w], linv[0:1, 0:w])
        scaled = post.tile([NHID, MY], F32, tag="scaled")
        nc.vector.tensor_mul(scaled[:, 0:w], aggps[0:NHID, c0:c1],
                             lbs[:, 0:w])
        # ELU(x)+1 = max(x,0) + exp(min(x,0)); min via relu(-x) on ACT
        nr = post.tile([NHID, MY], F32, tag="tmp")
        nc.scalar.activation(nr[:, 0:w], scaled[:, 0:w], AF.Relu, scale=-1.0)
        em = post.tile([NHID, MY], F32, tag="tmp")
        nc.scalar.activation(em[:, 0:w], nr[:, 0:w], AF.Exp, scale=-1.0)
        dst_rows = hcatT[64 * (h % 2):64 * (h % 2) + NHID, h // 2, c0:c1]
        nc.vector.scalar_tensor_tensor(dst_rows, scaled[:, 0:w], 0.0,
                                       em[:, 0:w], op0=ALU.max, op1=ALU.add)
        if h % 2 == 1:
            t = h // 2
            nc.tensor.matmul(wh2ps[:, c0:c1], wv2r[:, t, :],
                             hcatT[:, t, c0:c1], start=False, stop=(t == 3))

    pending = None
    head_prefix(0)
    head_prefix(1)
    for h in range(H):
        def cb(p=pending, h=h):
            if p is not None:
                l1_epilogue(*p)
            if h + 2 < H:
                head_prefix(h + 2)
        aggps = attention(
            embsb[:, h, :],
            lambda jc, h=h: e02dc[:, jc, h:h + 1],
            lambda jc, h=h: e1dc[:, jc, h:h + 1],
            lambda jc, h=h: ne1dc[:, jc, h:h + 1],
            lambda jc, h=h: whca[:, jc, h, :],
            lambda jc, h=h: whce[:, M_IDX[jc], h, :], interleave=cb,
            pre_chunk=(whca_chunk if h == 0 else None),
            pattern=(PATTERN0 if h == 0 else PATTERN))
        pending = (h, aggps)
    # ============ PHASE D: output attention layer ========================
    # The final head's epilogue, Wh2 matmul, payload transpose and gather
    # run per column half so slice 0's AllGather flies while half B still
    # computes.  wh2Tme rows 0:64 = Wh2 (f32r); row 64 = raw dst2 (c2 bias
    # folded into the post-gather exps).  The L2 attention runs in f32r:
    # batchnorm divides by a tiny cross-node std, amplifying L2-side noise,
    # so bf16 is not enough here.
    p2p = ctx.enter_context(tc.tile_pool(name="p2p", bufs=1, space="PSUM"))
    wh2Tme = p2.tile([NOUT + 1, MY], F32)
    payload = p2.tile([128, NIB, NOUT + 1], F32R)
    cc_in = dram.tile([2, 256, NOUT + 1], F32R)
    cc_out = dram.tile([2, N_CORES, 256, NOUT + 1], F32R)
    h7, agg7 = pending
    for s in range(2):
        c0, c1 = 256 * s, 256 * s + 256
        l1_epilogue(h7, agg7, c0, c1)
        nc.vector.tensor_copy(wh2Tme[:, c0:c1], wh2ps[0:NOUT + 1, c0:c1])
        for tl in range(2):
            t = 2 * s + tl
            pps = p2p.tile([128, 512], F32, tag="pp2")
            nc.tensor.transpose(pps[:, 0:NOUT + 1],
                                wh2Tme[:, 128 * t:128 * (t + 1)],
                                ident[0:NOUT + 1, 0:NOUT + 1])
            if tl:
                nc.scalar.copy(payload[:, t, :], pps[:, 0:NOUT + 1])
            else:
                nc.vector.tensor_copy(payload[:, t, :], pps[:, 0:NOUT + 1])
        nc.sync.dma_start(
            out=cc_in[s].rearrange("(t p) d -> p t d", t=2),
            in_=payload[:, 2 * s:2 * s + 2, :])
        collective("AllGather", ALU.bypass, cc_in[s], cc_out[s])

    # local-only src2 path (off the gather critical path)
    s2r = p2.tile([1, MY], F32R)
    nc.scalar.activation(s2r, wh2ps[96:97, :], AF.Exp,
                         scale=-0.8, bias=c2rep[0:1, 2:3])
    embsb2 = p2.tile([128, MY], F32R)
    nc.gpsimd.partition_broadcast(embsb2, s2r[0:1, :])

    # wh2aug[jp, jc=4r+t, 0:64] = Wh2 lhsT, col 64 = dst2 raw; f32r straight
    # from DMA (PE rounds on read; no conversion copies needed)
    wh2aug = p2.tile([128, NJC, NOUT + 1], F32R)
    wh2aug_r = wh2aug.rearrange("p (r t) d -> p r t d", t=NIB)
    e02d2 = p2.tile([128, NIB, 8], F32)
    e1d2 = p2.tile([128, NIB, 8], F32)
    for s in range(2):
        for tl in range(2):
            t = 2 * s + tl
            nc.scalar.dma_start(
                out=wh2aug_r[:, :, t, :],
                in_=cc_out[s][:, 128 * tl:128 * (tl + 1), :]
                .rearrange("r p d -> p r d"))
        dcol = wh2aug_r[:, :, 2 * s:2 * s + 2, NOUT:NOUT + 1]
        nc.scalar.activation(
            e02d2[:, 2 * s:2 * s + 2, :],
            dcol.rearrange("p r t one -> p t (r one)"),
            AF.Exp, scale=0.2, bias=c2rep[:, 0:1])
        nc.scalar.activation(
            e1d2[:, 2 * s:2 * s + 2, :],
            dcol.rearrange("p r t one -> p t (r one)"),
            AF.Exp, bias=c2rep[:, 1:2])
        # ones column for the denominator row of every chunk of this slice
        nc.vector.tensor_scalar(dcol, dcol, 0.0, 1.0,
                                op0=ALU.mult, op1=ALU.add)

    # L2 attention: chunks in slice-arrival order; pattern 'x' (ACT mul +
    # DVE stt) or 's' (Pool ts + Pool tt, no DVE/ACT)
    aggps2 = agg.tile([NHID + 1, MY], F32, tag="aggps")
    n_emitted = [0]
    for s in range(2):
        xs = [(r, t) for t in (2 * s, 2 * s + 1)
              for r in range(N_CORES) if r not in L2_S_ROWS]
        ss = [(r, t) for t in (2 * s, 2 * s + 1)
              for r in range(N_CORES) if r in L2_S_ROWS]
        pairs = []
        for i in range(len(xs)):
            if i % 3 == 0 and ss:
                pairs.append(ss.pop(0))
            pairs.append(xs[i])
        pairs.extend(ss)
        for r, t in pairs:
            jc = 4 * r + t
            k = n_emitted[0]
            n_emitted[0] += 1
            last = k == NJC - 1
            pt2 = hotf.tile([128, MY], F32R, tag="pt2")
            if r in L2_S_ROWS:
                m08f = hotf.tile([128, MY], F32, tag="rr")
                nc.gpsimd.tensor_scalar(m08f, embsb2, e02d2[:, t, r:r + 1],
                                        e1d2[:, t, r:r + 1],
                                        op0=ALU.mult, op1=ALU.max)
                nc.gpsimd.tensor_mul(pt2, m08f, adjT[:, jc, :])
            else:
                a2t = hotf.tile([128, MY], F32R, tag="rr")
                nc.scalar.mul(a2t, embsb2, e02d2[:, t, r:r + 1])
                nc.vector.scalar_tensor_tensor(pt2, a2t, e1d2[:, t, r:r + 1],
                                               adjT[:, jc, :],
                                               op0=ALU.max, op1=ALU.mult)
            nc.tensor.matmul(aggps2, wh2aug[:, jc, :], pt2[:],
                             start=(k == 0), stop=last)
    # sqrt-table preload: epst = sqrt(EPS^2) is a real set-3 ACT op whose
    # input is ready at t=0, pulling the table load off the post-collective
    # critical path
    epssq = p2.tile([NOUT, 1], F32)
    nc.gpsimd.memset(epssq, float(EPS) * float(EPS))
    epst = p2.tile([NOUT, 1], F32)
    with tc.high_priority(offset=70):
        nc.scalar.activation(epst, epssq, AF.Sqrt)

    # ============ PHASE E: batchnorm (single AllReduce of [S, S2]) =======
    linv2 = post.tile([1, MY], F32, tag="linv")
    nc.vector.reciprocal(linv2, aggps2[NHID:NHID + 1, :])
    lbs2 = post.tile([NOUT, MY], F32, tag="lbs")
    nc.gpsimd.partition_broadcast(lbs2, linv2[0:1, :])
    bnS = p2.tile([NOUT, 2], F32)
    out2n = p2.tile([NOUT, MY], F32)
    nc.vector.scalar_tensor_tensor(out2n, aggps2[0:NOUT, :], 0.0, lbs2,
                                   op0=ALU.add, op1=ALU.mult,
                                   accum_out=bnS[:, 0:1])
    sq = post.tile([NOUT, MY], F32, tag="tmp")
    nc.vector.scalar_tensor_tensor(sq, out2n, 0.0, out2n,
                                   op0=ALU.add, op1=ALU.mult,
                                   accum_out=bnS[:, 1:2])
    bn_in = dram.tile([NOUT, 2], F32)
    bn_out = dram.tile([NOUT, 2], F32)
    nc.scalar.dma_start(out=bn_in, in_=bnS)
    collective("AllReduce", ALU.add, bn_in, bn_out)
    bnG = p2.tile([NOUT, 2], F32)
    nc.scalar.dma_start(out=bnG, in_=bn_out)
    negmu = p2.tile([NOUT, 1], F32)
    nc.vector.tensor_scalar_mul(negmu, bnG[:, 0:1], -1.0 / N)
    mu2 = p2.tile([NOUT, 1], F32)
    nc.vector.tensor_mul(mu2, negmu, negmu)
    var = p2.tile([NOUT, 1], F32)
    nc.vector.scalar_tensor_tensor(var, bnG[:, 1:2], 1.0 / N, mu2,
                                   op0=ALU.mult, op1=ALU.subtract)

    sd = p2.tile([NOUT, 1], F32)
    nc.scalar.activation(sd, var, AF.Sqrt, bias=epst[:, 0:1])
    rstd = p2.tile([NOUT, 1], F32)
    nc.vector.reciprocal(rstd, sd)
    scale = p2.tile([NOUT, 1], F32)
    nc.vector.tensor_mul(scale, rstd, gb[:, 0:1])
    shift = p2.tile([NOUT, 1], F32)
    nc.vector.tensor_mul(shift, negmu, scale)
    nc.vector.tensor_add(shift, shift, gb[:, 1:2])

    finT = p2.tile([NOUT, MY], F32)
    nc.scalar.activation(finT, out2n, AF.Tanh, bias=shift[:, 0:1],
                         scale=scale[:, 0:1])
    nc.sync.dma_start(out=d["y"].ap(), in_=finT)


def _prep_inputs(x, adj, W_heads, a_heads, W_out, a_out, gamma, beta):
    """Host-side packing of the small weights + per-core sharding."""
    import ml_dtypes
    BF = ml_dtypes.bfloat16
    x = np.ascontiguousarray(np.asarray(x, dtype=np.float32))
    adj = np.asarray(adj, dtype=np.int32)
    W_heads = np.asarray(W_heads, dtype=np.float32)
    a_heads = np.asarray(a_heads, dtype=np.float32)
    W_out = np.asarray(W_out, dtype=np.float32)
    a_out = np.asarray(a_out, dtype=np.float32)

    wall = np.ascontiguousarray(
        W_heads.transpose(1, 0, 2).reshape(NIN, H * NHID))
    vall = np.zeros((NIN, 2 * H), np.float32)
    for h in range(H):
        vall[:, h] = W_heads[h] @ a_heads[h, NHID:]        # dst direction
        vall[:, H + h] = W_heads[h] @ a_heads[h, :NHID]    # src direction
    # woutt[p, t*64+d] = W_out[t*128+p, d]  (lhsT chunks)
    woutt = W_out.reshape(4, 128, NOUT).transpose(1, 0, 2).reshape(128, -1)
    wcs = W_out.sum(axis=0)
    v2d = W_out @ a_out[NOUT:]          # dst direction, [512]
    v2s = W_out @ a_out[:NOUT]          # src direction
    v2t = (np.stack([v2d, v2s], axis=1).reshape(4, 128, 2)
           .transpose(1, 0, 2).reshape(128, 8))
    eye = np.eye(128, dtype=np.float32)
    c2d = -(wcs @ a_out[NOUT:])
    c2s = -(wcs @ a_out[:NOUT])

    blobf = np.zeros((128, 624), np.float32)
    blobf[:, 0:128] = eye
    blobf[0:NOUT, 128] = np.asarray(gamma, np.float32)
    blobf[0:NOUT, 129] = np.asarray(beta, np.float32)
    blobf[:, 130] = 0.2 * c2d
    blobf[:, 131] = c2d
    blobf[:, 132] = -0.8 * c2s
    # packed [W_out | v2d | 0.. | v2s] per 128-row chunk t
    wv2 = np.zeros((4, 128, 97), np.float32)
    wv2[:, :, 0:NOUT] = W_out.reshape(4, 128, NOUT)
    wv2[:, :, NOUT] = v2d.reshape(4, 128)
    wv2[:, :, 96] = v2s.reshape(4, 128)
    blobf[:, 133:521] = wv2.transpose(1, 0, 2).reshape(128, -1)
    blobf[0, 521:585] = -wcs
    blobb = np.zeros((128, 528), np.float32)
    blobb[:, 0:512] = wall
    blobb[:, 512:528] = vall
    blobb = blobb.astype(BF)

    xt = np.ascontiguousarray(x.T).astype(BF)
    adjbf = adj.astype(BF)
    shared = {"xt": xt, "blobf": blobf, "blobb": blobb}
    in_maps = []
    for c in range(N_CORES):
        m = dict(shared)
        m["xmt"] = np.ascontiguousarray(x[MY * c:MY * (c + 1)].T).astype(BF)
        # adjt[p, jc*MY + i] = adj[MY*c + i, 128*jc + p]
        m["adjt"] = np.ascontiguousarray(
            adjbf[MY * c:MY * (c + 1)].T.reshape(NJC, 128, MY)
            .transpose(1, 0, 2).reshape(128, NJC * MY))
        in_maps.append(m)
    return in_maps


def kernel(x, adj, W_heads, a_heads, W_out, a_out, gamma, beta, **kw):
    if "nc" not in _CACHED:
        _CACHED["nc"] = build_program()
    nc = _CACHED["nc"]
    in_maps = _prep_inputs(x, adj, W_heads, a_heads, W_out, a_out, gamma, beta)
    res = run_bass_kernel_spmd(nc, in_maps, core_ids=list(range(N_CORES)),
                               **kw)
    _CACHED["last_res"] = res
    out = np.concatenate([res.results[c]["y"].T for c in range(N_CORES)],
                         axis=0)
    return np.ascontiguousarray(out)
